# revision 1
# baseline (speedup 1.0000x reference)
# Trainium2 Bass kernel for nn_FDM_3899830304921 (feature-map cosine-sim
# dual-softmax transport), data-parallel over batch on 8 NeuronCores.
#
# v2.1: fp8e4 DoubleRow matmuls; r2-normalization folded into the f2
# quantize; exp reads gram PSUM directly; E stored shifted (E-1) in fp8
# with exact rank-1 corrections folded into the output-scale pass
# (cancels the fp8 cancellation-noise floor: ~4e-3 vs ~4e-2 unshifted);
# Newton rsqrt on DVE (zero ACT table switches); ET via fp8 PE transpose
# with colsum on the ACT copy accumulator; broadcasts via 1-partition
# PE matmuls.
#
# Math per batch (c=512, n=m=784):
#   f1q  = fp8(f1)            [c,n]  (+ S1[c]=sum_n f1 via accum, fp32)
#   f1T  = fp8(T(f1q))        [n,c]  (+ ssq1[n] via fp8 squares)
#   f2T  = fp8(T(f2))         [m,c]  (+ ssq2[m] via f32 squares)
#   r1=1/sqrt(ssq1), r2=1/sqrt(ssq2)   (Newton on DVE)
#   f2n  = fp8(-16*r2[m]*f2)  [c,m]
#   G'   = f1q^T @ f2n        [n,m]  (= -16*cos*||f1q_:n||)
#   E    = exp(G'*r1/16) bf16, rowsum rs via accum
#   E'   = fp8(E - 1);  ET' = T(E'), colsum cs' via copy accum
#   S2[c]= sum_m f2  (exact fp32)
#   o2   = (f1T^T @ E' + S1) * (.001/colsum)
#   o1   = (f2T^T @ ET' + S2) * (.001/rowsum)
import sys

if "/opt/trn_rl_repo" not in sys.path:
    sys.path.insert(0, "/opt/trn_rl_repo")

import numpy as np

B_TOTAL = 32
B_PER_CORE = 4
N_CORES = 8
C = 512
N = 784  # 28*28, both spatial dims
FACTOR = 0.001
RSQRT_SEED = 0.044194173824159216  # 1/sqrt(512)

# n (and m) tiling: 6 tiles of 128 + one of 16
NT = [(0, 128), (128, 128), (256, 128), (384, 128), (512, 128), (640, 128), (768, 16)]
# free-dim split of 784 into PSUM-bank-sized pieces
HALVES = [(0, 512), (512, 272)]

_BUILT = {}


def _build(nbatch, enable_asserts=False):
    key = (nbatch, enable_asserts)
    if key in _BUILT:
        return _BUILT[key]

    import concourse.bass as bass
    import concourse.tile as tile
    from concourse import bacc, mybir
    from concourse.masks import make_identity

    f32 = mybir.dt.float32
    f32r = mybir.dt.float32r
    f8 = mybir.dt.float8e4
    bf16 = mybir.dt.bfloat16
    AF = mybir.ActivationFunctionType
    ALU = mybir.AluOpType
    DR = mybir.MatmulPerfMode.DoubleRow

    nc = bacc.Bacc("TRN2", target_bir_lowering=False, debug=False,
                   enable_asserts=enable_asserts, num_devices=N_CORES)
    fm1 = nc.dram_tensor("fm1", [nbatch, C, N], f32, kind="ExternalInput").ap()
    fm2 = nc.dram_tensor("fm2", [nbatch, C, N], f32, kind="ExternalInput").ap()
    o1 = nc.dram_tensor("o1", [nbatch, C, N], f32, kind="ExternalOutput").ap()
    o2 = nc.dram_tensor("o2", [nbatch, C, N], f32, kind="ExternalOutput").ap()

    with tile.TileContext(nc) as tc:
        with (
            tc.tile_pool(name="sb", bufs=2) as sb,
            tc.tile_pool(name="ps", bufs=2, space="PSUM") as ps,
            tc.tile_pool(name="dr", bufs=2, space="DRAM") as dram,
        ):
            identf = sb.tile([128, 128], f32, tag="identf", bufs=1)
            make_identity(nc, identf[:])
            ident8 = sb.tile([128, 128], f8, tag="ident8", bufs=1)
            nc.scalar.copy(ident8[:], identf[:])
            identb = sb.tile([128, 128], bf16, tag="identb", bufs=1)
            nc.gpsimd.tensor_copy(out=identb[:], in_=identf[:])
            identr = sb.tile([128, 128], f32r, tag="identr", bufs=1)
            nc.scalar.copy(identr[:], identf[:])
            onesb = sb.tile([1, 128], bf16, tag="onesb", bufs=1)
            nc.vector.memset(onesb[:], 1.0)
            onesB = sb.tile([128, N], bf16, tag="onesB", bufs=1)
            nc.gpsimd.memset(onesB[:], 1.0)
            ones8 = sb.tile([128, 8, 16], f8, tag="ones8", bufs=1)
            nc.vector.memset(ones8[:], 1.0)
            # constants for the sum-row broadcasts: out = 1000*(sum' + N)
            kcolf = sb.tile([1, 128], f32, tag="kcolf", bufs=1)
            nc.vector.memset(kcolf[:], 1000.0)
            kcol = sb.tile([1, 128], f32r, tag="kcol", bufs=1)
            nc.scalar.copy(kcol[:], kcolf[:])
            kNcolf = sb.tile([1, 128], f32, tag="kNcolf", bufs=1)
            nc.vector.memset(kNcolf[:], 784000.0)
            kNcol = sb.tile([1, 128], f32r, tag="kNcol", bufs=1)
            nc.scalar.copy(kNcol[:], kNcolf[:])
            onerowf = sb.tile([1, N], f32, tag="onerowf", bufs=1)
            nc.vector.memset(onerowf[:], 1.0)
            onerow = sb.tile([1, N], f32r, tag="onerow", bufs=1)
            nc.scalar.copy(onerow[:], onerowf[:])

            def colrecip_bcast(colsum, extra, rowtag, width=8, colstep=1):
                """cols [128,width] f32 sums -> bcast of 0.001/(sum+extra)."""
                rcf = sb.tile([128, width], f32, tag=rowtag + "_f", bufs=2)
                rcb = sb.tile([128, width], bf16, tag=rowtag + "_c", bufs=2)
                with nc.allow_low_precision(reason="softmax scale rows"):
                    nc.vector.tensor_scalar(
                        out=rcf[:], in0=colsum, scalar1=1000.0,
                        scalar2=1000.0 * extra, op0=ALU.mult, op1=ALU.add)
                    nc.vector.reciprocal(rcf[:], rcf[:])
                    nc.vector.tensor_scalar(
                        out=rcb[:], in0=rcf[:], scalar1=1.0, scalar2=None,
                        op0=ALU.mult)
                row = col_to_row(rcb, "s", colstep=colstep)
                d = dram.tile([1, N], bf16, tag=rowtag + "_d", bufs=2)
                nc.sync.dma_start(out=d[:], in_=row[:1, :])
                dap = d[:]
                srcap = bass.AP(tensor=dap.tensor, offset=dap.offset,
                                ap=[[0, 128]] + list(dap.ap))
                out = sb.tile([128, N], bf16, tag=rowtag + "_B", bufs=2)
                nc.sync.dma_start(
                    out=out[:].rearrange("p (a x) -> p a x", a=1), in_=srcap)
                return out

            def bcastb(rowtile):
                """[1, N] bf16 SBUF row -> [128, N] f32 PSUM via 1-part mm."""
                out = ps.tile([128, N], f32, tag="bc", bufs=1)
                for hoff, hsz in HALVES:
                    nc.tensor.matmul(
                        out[:, hoff:hoff + hsz],
                        onesb[:1, :],
                        rowtile[:1, hoff:hoff + hsz],
                        start=True, stop=True)
                return out

            def f8ps(ptf, col0, ncols, rows=128):
                a = ptf[:].bitcast(f8)
                return bass.AP(tensor=a.tensor, offset=a.offset + 2 * col0,
                               ap=[list(a.ap)[0], [2, ncols]])[:rows]

            def col_to_row(coltile, eng, colstep=1):
                """[128, 8] bf16 cols -> [1, N] bf16 SBUF row."""
                prt = ps.tile([128, N], f32, tag="big", bufs=4)
                pr = prt[:].bitcast(bf16)
                for t, (noff, nsz) in enumerate(NT):
                    nc.tensor.transpose(
                        pr[:1, noff:noff + nsz],
                        coltile[:nsz, colstep * t:colstep * t + 1],
                        identb[:nsz, :nsz])
                row = sb.tile([1, N], bf16, tag="row", bufs=3)
                if eng == "v":
                    nc.vector.tensor_copy(out=row[:1, :], in_=pr[:1, :N])
                else:
                    nc.scalar.copy(row[:1, :], pr[:1, :N])
                return row

            def load(b):
                """Queue input loads for batch b on the SP DMA queue."""
                f1_sb = sb.tile([128, 4, N], f32, tag="f1", bufs=3)
                nc.sync.dma_start(
                    out=f1_sb[:],
                    in_=fm1[b].rearrange("(t p) n -> p t n", p=128))
                f2_sb = sb.tile([128, 4, N], f32r, tag="f2", bufs=3)
                nc.sync.dma_start(
                    out=f2_sb[:],
                    in_=fm2[b].rearrange("(t p) n -> p t n", p=128).bitcast(f32r))
                return f1_sb, f2_sb

            def h1(b):
                f1_sb, f2_sb = load(b)

                # ---- quantize f1 -> fp8 + exact S1 accum (GP, first in GP stream
                # so PE's T(f1q) isn't blocked behind the f2T copies) ----
                f1q = sb.tile([128, 4, N], f8, tag="f1q", bufs=2)
                s1 = sb.tile([128, 4], f32, tag="s1", bufs=2)
                for j in range(4):
                    nc.vector.tensor_scalar(
                        out=f1q[:, j, :], in0=f1_sb[:, j, :],
                        scalar1=1.0, scalar2=0.0, op0=ALU.mult, op1=ALU.add,
                        accum_out=s1[:, j:j + 1])

                # ---- S2[c] = sum_m f2 (exact, ACT accum; junk out) ----
                s2 = sb.tile([128, 4], f32, tag="s2", bufs=2)
                junkb = sb.tile([128, N], bf16, tag="junkb", bufs=2)
                for j in range(4):
                    nc.scalar.activation(
                        out=junkb[:], in_=f2_sb[:, j, :].bitcast(f32),
                        func=AF.Copy, accum_out=s2[:, j:j + 1])

                # ---- T(f2) f32r; f2T fp8 copies (GP); ssq2 squares (DVE) ----
                ssq1 = sb.tile([128, 16], f32, tag="ssq1", bufs=2)
                nc.vector.memset(ssq1[:], 1.0)
                f2T = sb.tile([128, 8, C], f8, tag="f2T", bufs=2)
                nc.gpsimd.memset(f2T[:, 6:8, :], 0)
                junkv = sb.tile([128, C], bf16, tag="junkv", bufs=2)
                for t, (noff, nsz) in enumerate(NT):
                    ptf = ps.tile([128, N], f32, tag="big", bufs=4)
                    pt = ptf[:, :C]
                    for j in range(4):
                        nc.tensor.transpose(
                            pt[:nsz, j * 128:(j + 1) * 128].bitcast(f32r),
                            f2_sb[:, j, noff:noff + nsz],
                            identr[:, :])
                    nc.vector.tensor_copy(out=f2T[:nsz, t, :], in_=pt[:nsz, :])
                    nc.scalar.activation(
                        out=junkv[:nsz], in_=pt[:nsz, :], func=AF.Square,
                        accum_out=ssq1[:nsz, 8 + t:9 + t])

                def newton(dst_lo, dst_hi):
                    yt = sb.tile([128, 8], f32, tag=f"y{dst_lo}", bufs=2)
                    ya = sb.tile([128, 8], f32, tag=f"ya{dst_lo}", bufs=2)
                    nc.vector.memset(yt[:], RSQRT_SEED)
                    u = ssq1[:, dst_lo:dst_hi]
                    for it in range(3):
                        nc.vector.tensor_tensor(out=ya[:], in0=yt[:], in1=yt[:],
                                                op=ALU.mult)
                        nc.vector.tensor_tensor(out=ya[:], in0=ya[:], in1=u,
                                                op=ALU.mult)
                        nc.vector.tensor_scalar(
                            out=ya[:], in0=ya[:], scalar1=-0.5, scalar2=1.5,
                            op0=ALU.mult, op1=ALU.add)
                        nc.vector.tensor_tensor(out=yt[:], in0=yt[:], in1=ya[:],
                                                op=ALU.mult)
                    return yt

                # r2 chain immediately (hides under T(f1q) PE work)
                y2 = newton(8, 16)
                r2b = sb.tile([128, 8], bf16, tag="r2b", bufs=2)
                with nc.allow_low_precision(reason="bf16 r2 row"):
                    nc.vector.tensor_scalar(
                        out=r2b[:], in0=y2[:], scalar1=1.0, scalar2=None,
                        op0=ALU.mult)
                r2row = col_to_row(r2b, "s")
                d2 = dram.tile([1, N], bf16, tag="r2d", bufs=2)
                nc.sync.dma_start(out=d2[:], in_=r2row[:1, :])
                dap = d2[:]
                srcap = bass.AP(tensor=dap.tensor, offset=dap.offset,
                                ap=[[0, 128]] + list(dap.ap))
                r2B = sb.tile([128, N], bf16, tag="r2B", bufs=2)
                nc.sync.dma_start(
                    out=r2B[:].rearrange("p (a x) -> p a x", a=1), in_=srcap)
                f2n = sb.tile([128, 4, N], f8, tag="f2n", bufs=2)
                for j in range(4):
                    nc.vector.scalar_tensor_tensor(
                        out=f2n[:, j, :], in0=f2_sb[:, j, :].bitcast(f32), scalar=-16.0,
                        in1=r2B[:, :], op0=ALU.mult, op1=ALU.mult)

                # ---- T(f1q) fp8 -> f1T; ssq1 squares (ACT) ----
                f1T = sb.tile([128, 8, C], f8, tag="f1T", bufs=2)
                nc.gpsimd.memset(f1T[:, 6:8, :], 0)
                junk = sb.tile([128, C], bf16, tag="junk", bufs=2)
                for t, (noff, nsz) in enumerate(NT):
                    ptf = ps.tile([128, N], f32, tag="big", bufs=4)
                    for j in range(4):
                        nc.tensor.transpose(
                            f8ps(ptf, j * 128, 128, nsz),
                            f1q[:, j, noff:noff + nsz],
                            ident8[:, :])
                    nc.vector.tensor_copy(out=f1T[:nsz, t, :],
                                          in_=f8ps(ptf, 0, C, nsz))
                    nc.scalar.activation(
                        out=junk[:nsz], in_=f8ps(ptf, 0, C, nsz),
                        func=AF.Square, accum_out=ssq1[:nsz, t:t + 1])

                # r1 chain (runs during gram; needed first at exp t0)
                y1 = newton(0, 8)
                r1s = sb.tile([128, 8], f32, tag="r1s", bufs=2)
                nc.vector.tensor_scalar(
                    out=r1s[:], in0=y1[:], scalar1=0.0625, scalar2=None,
                    op0=ALU.mult)

                # ---- gram G' = f1q^T @ f2n (DR) + exp -> E bf16 ----
                E = sb.tile([128, 7, N], bf16, tag="E", bufs=2)
                rsc = sb.tile([128, 8], f32, tag="rsc", bufs=2)
                nc.vector.memset(rsc[:], 1.0)
                for t, (noff, nsz) in enumerate(NT):
                    G = ps.tile([128, N], f32, tag="big", bufs=4)
                    for k in range(2):
                        for hoff, hsz in HALVES:
                            nc.tensor.matmul(
                                G[:nsz, hoff:hoff + hsz],
                                f1q[:, 2 * k:2 * k + 2, noff:noff + nsz],
                                f2n[:, 2 * k:2 * k + 2, hoff:hoff + hsz],
                                start=(k == 0), stop=(k == 1), perf_mode=DR)
                    nc.scalar.activation(
                        out=E[:nsz, t, :], in_=G[:nsz, :], func=AF.Exp,
                        scale=r1s[:nsz, t:t + 1],
                        accum_out=rsc[:nsz, t:t + 1])

                # ---- shift: E' = fp8(E - 1) (DVE, one 5488-el instr) ----
                Ep = sb.tile([128, 8, N], f8, tag="Ep", bufs=2)
                nc.gpsimd.memset(Ep[:, 6:8, :], 0)
                with nc.allow_low_precision(reason="fp8 shifted E"):
                    for t in range(6):
                        nc.vector.tensor_scalar(
                            out=Ep[:, t, :], in0=E[:, t, :], scalar1=-1.0,
                            scalar2=None, op0=ALU.add)
                    nc.vector.tensor_scalar(
                        out=Ep[:16, 6, :], in0=E[:16, 6, :], scalar1=-1.0,
                        scalar2=None, op0=ALU.add)


                return f1T, f2T, Ep, s1, s2, rsc

            def h2(b, f1T, f2T, Ep, s1, s2, rsc):
                # ---- colsum'(E')[m] via Ep-stationary DR mms: [msz,16]
                # blocks per m-tile (col 0..15 all equal the sum) ----
                pcf = ps.tile([128, N], f32, tag="big", bufs=4)
                nc.vector.memset(pcf[:, 96:112], 1.0)
                for t, (moff, msz) in enumerate(NT):
                    c0 = 16 * t
                    for u in range(4):
                        nc.tensor.matmul(
                            pcf[:msz, c0:c0 + 16],
                            Ep[:, 2 * u:2 * u + 2, moff:moff + msz],
                            ones8[:, 2 * u:2 * u + 2, :],
                            start=(u == 0), stop=(u == 3), perf_mode=DR)
                rcB = colrecip_bcast(pcf[:, 0:112], float(N), "rc",
                                     width=112, colstep=16)

                def out_mm(dst, statT, mov, scol, sclB):
                    """dst[b, c, :] = (statT^T @ mov + scol) * sclB."""
                    for ci in range(4):
                        csl = slice(ci * 128, (ci + 1) * 128)
                        P = ps.tile([128, N], f32, tag="big", bufs=4)
                        for u in range(4):
                            for hoff, hsz in HALVES:
                                nc.tensor.matmul(
                                    P[:, hoff:hoff + hsz],
                                    statT[:, 2 * u:2 * u + 2, csl],
                                    mov[:, 2 * u:2 * u + 2, hoff:hoff + hsz],
                                    start=(u == 0), stop=(u == 3), perf_mode=DR)
                        O = sb.tile([128, N], f32, tag="o", bufs=4)
                        nc.vector.scalar_tensor_tensor(
                            out=O[:], in0=P[:], scalar=scol[:, ci:ci + 1],
                            in1=sclB[:, :], op0=ALU.add, op1=ALU.mult)
                        nc.sync.dma_start(out=dst[b, csl, :], in_=O[:])

                # out2 needs only E' + rcB (no TE dependency)
                rrB = colrecip_bcast(rsc, 0.0, "rr")
                out_mm(o2, f1T, Ep, s1, rcB)

                # ---- ET' = T(E') fp8, plain ACT copies (no accum) ----
                ET = sb.tile([128, 8, N], f8, tag="ET", bufs=2)
                nc.gpsimd.memset(ET[:, 6:8, :], 0)
                for t, (moff, msz) in enumerate(NT):
                    pmf = ps.tile([128, N], f32, tag="big", bufs=4)
                    for u, (noff, nsz) in enumerate(NT):
                        nc.tensor.transpose(
                            f8ps(pmf, noff, nsz, msz),
                            Ep[:nsz, u, moff:moff + msz],
                            ident8[:nsz, :nsz])
                    nc.scalar.copy(ET[:msz, t, :], f8ps(pmf, 0, N, msz))

                out_mm(o1, f2T, ET, s2, rrB)


            prev = h1(0)
            for b in range(nbatch):
                nxt = h1(b + 1) if b + 1 < nbatch else None
                h2(b, *prev)
                prev = nxt

    nc.compile()
    _BUILT[key] = nc
    return nc


def _run(fm1, fm2, trace=False):
    from concourse.bass_utils import run_bass_kernel_spmd

    fm1 = np.ascontiguousarray(np.asarray(fm1, np.float32).reshape(B_TOTAL, C, N))
    fm2 = np.ascontiguousarray(np.asarray(fm2, np.float32).reshape(B_TOTAL, C, N))
    nc = _build(B_PER_CORE)
    f1s = fm1.reshape(N_CORES, B_PER_CORE, C, N)
    f2s = fm2.reshape(N_CORES, B_PER_CORE, C, N)
    in_maps = [
        {"fm1": np.ascontiguousarray(f1s[i]), "fm2": np.ascontiguousarray(f2s[i])}
        for i in range(N_CORES)
    ]
    res = run_bass_kernel_spmd(nc, in_maps, core_ids=list(range(N_CORES)),
                               trace=trace)
    out1 = np.concatenate([res.results[i]["o1"] for i in range(N_CORES)], axis=0)
    out2 = np.concatenate([res.results[i]["o2"] for i in range(N_CORES)], axis=0)
    out1 = out1.reshape(B_TOTAL, C, 28, 28).astype(np.float32)
    out2 = out2.reshape(B_TOTAL, C, 28, 28).astype(np.float32)
    return (out1, out2), res


def kernel(fm1, fm2):
    (out1, out2), _ = _run(fm1, fm2)
    return out1, out2

